# revision 29
# baseline (speedup 1.0000x reference)
"""AdaFuse MoE-routing kernel for 8 Trainium2 NeuronCores (Bass/Tile).

Strategy (pure data parallel over batch B=8192, 1024 rows per core):
  - All activations kept FEATURE-MAJOR on-chip ([feat_part, batch_free]) so
    every Linear layer is out = W_T.T @ act with no per-layer transposes;
    host passes inputs/weights pre-transposed (layout choice only).
  - Matmuls run in bf16 (f32 PSUM accumulation); elementwise work in f32/bf16.
  - Each core normalizes its target shard, casts bf16, AllGathers the
    [768,1024] shard -> [6144,1024] (rank-major blocks), then computes its
    [1024, 8192] logits block streaming gathered tiles.
  - The output normalization (1/||out|| * exp(logit_scale)) is applied in the
    logits PSUM->SBUF epilogue as a per-partition scale, so logits matmuls
    depend only on the unnormalized output tiles.
  - Weight loads / gathered-target loads / logits stores are batched into
    wide DMAs (DMA-issue on the sequencers costs ~0.7us per descriptor).

Self-contained: hardcodes all shapes; no file reads.
"""
import sys

sys.path.insert(0, "/opt/trn_rl_repo")

import numpy as np

F_DIM, P_DIM, H_DIM, N_EXP = 768, 512, 1024, 4
B, T = 8192, 8192
N_CORES = 8
BS = B // N_CORES            # 1024 batch rows per core
TS = T // N_CORES            # 1024 target rows per core
FT = F_DIM // 128            # 6 feature tiles
PT2 = (2 * P_DIM) // 128     # 8 comb-dim tiles
HT = H_DIM // 128            # 8 hidden tiles
NH = BS // 512               # 2 free-dim halves per batch


def build():
    from concourse import bacc, tile, mybir

    f32 = mybir.dt.float32
    bf16 = mybir.dt.bfloat16
    AF = mybir.ActivationFunctionType
    OP = mybir.AluOpType

    nc = bacc.Bacc(None, target_bir_lowering=False, debug=False, num_devices=N_CORES)

    # ---- kernel I/O ----
    tT = nc.dram_tensor("tT", [F_DIM, BS], f32, kind="ExternalInput")
    iT = nc.dram_tensor("iT", [F_DIM, BS], f32, kind="ExternalInput")
    gT = nc.dram_tensor("gT", [F_DIM, TS], f32, kind="ExternalInput")
    WtT = nc.dram_tensor("WtT", [F_DIM, P_DIM], f32, kind="ExternalInput")
    WiT = nc.dram_tensor("WiT", [F_DIM, P_DIM], f32, kind="ExternalInput")
    bt = nc.dram_tensor("bt", [P_DIM, 1], f32, kind="ExternalInput")
    bi = nc.dram_tensor("bi", [P_DIM, 1], f32, kind="ExternalInput")
    rtW1T = nc.dram_tensor("rtW1T", [2 * P_DIM, H_DIM // 4], f32, kind="ExternalInput")
    rtb1 = nc.dram_tensor("rtb1", [H_DIM // 4, 1], f32, kind="ExternalInput")
    rtW2T = nc.dram_tensor("rtW2T", [H_DIM // 4, N_EXP], f32, kind="ExternalInput")
    rtb2 = nc.dram_tensor("rtb2", [N_EXP, 1], f32, kind="ExternalInput")
    dsW1T = nc.dram_tensor("dsW1T", [2 * P_DIM, H_DIM], f32, kind="ExternalInput")
    dsb1 = nc.dram_tensor("dsb1", [H_DIM, 1], f32, kind="ExternalInput")
    dsW2T = nc.dram_tensor("dsW2T", [H_DIM, 1], f32, kind="ExternalInput")
    dsb2 = nc.dram_tensor("dsb2", [1, 1], f32, kind="ExternalInput")
    expWT = nc.dram_tensor("expWT", [N_EXP, 2 * P_DIM, H_DIM], f32, kind="ExternalInput")
    expb = nc.dram_tensor("expb", [N_EXP * H_DIM, 1], f32, kind="ExternalInput")
    outWT = nc.dram_tensor("outWT", [H_DIM, F_DIM], f32, kind="ExternalInput")
    outb = nc.dram_tensor("outb", [F_DIM, 1], f32, kind="ExternalInput")
    lam = nc.dram_tensor("lam", [1, 1], f32, kind="ExternalInput")

    out_logits = nc.dram_tensor("out_logits", [BS, T], f32, kind="ExternalOutput")
    out_ds = nc.dram_tensor("out_ds", [1, BS], f32, kind="ExternalOutput")
    out_rp = nc.dram_tensor("out_rp", [N_EXP, BS], f32, kind="ExternalOutput")

    agin = nc.dram_tensor("agin", [F_DIM, TS], bf16, kind="Internal")
    agout = nc.dram_tensor("agout", [N_CORES * F_DIM, TS], bf16,
                           kind="Internal", addr_space="Shared")
    invbounce = nc.dram_tensor("invbounce", [1, HT * 128], f32, kind="Internal")

    ld = nc.sync       # input DMA queue (HWDGE)
    st = nc.scalar     # output / secondary DMA queue (HWDGE)

    with tile.TileContext(nc) as tc:
        with (
            tc.tile_pool(name="const", bufs=1) as constp,
            tc.tile_pool(name="pp", bufs=1) as pp,
            tc.tile_pool(name="wk", bufs=1) as wk,
            tc.tile_pool(name="ps", bufs=6, space="PSUM") as ps,
            tc.tile_pool(name="ps1", bufs=2, space="PSUM") as ps1,
        ):
            ones = constp.tile([128, 1], bf16)
            nc.vector.memset(ones[:], 1.0)
            def psum(tag="mm"):
                return ps.tile([128, 512], f32, tag=tag, bufs=6, name=tag)

            def psum1(nm):
                return ps1.tile([N_EXP, 512], f32, tag="ss", bufs=2, name=nm)

            def vec1(nm, dt=f32):
                return pp.tile([1, BS], dt, tag="vec1", bufs=2, name=nm)

            def bcast128(nm, dt=bf16):
                return pp.tile([128, BS], dt, tag="bcast", bufs=2, name=nm)

            def wbf16(nm, tag="wbf", bufs=2):
                return wk.tile([128, BS], bf16, tag=tag, bufs=bufs, name=nm)

            last_wcast = [None]

            def load_w2(dram2d, pair, n, nm):
                """Two adjacent [128, n] f32 k-tiles -> one wide bf16 tile.

                Returns tile whose cols [j*n:(j+1)*n] hold k-tile 2*pair+j.
                """
                w32 = wk.tile([128, 2048], f32, tag="w32", bufs=2, name="w32" + nm)
                src = dram2d[pair * 256:(pair + 1) * 256, 0:n]
                src = src.rearrange("(j p) c -> p j c", p=128)
                eng = ld if (pair % 2 == 0) else nc.gpsimd
                eng.dma_start(w32[:, 0:2 * n].rearrange("p (j c) -> p j c", j=2), src)
                w = pp.tile([128, 2048], bf16, tag="wbig", bufs=12, name=nm)
                if n >= 768:
                    ci = nc.vector.tensor_copy(w[:, 0:2 * n], w32[:, 0:2 * n])
                else:
                    ci = nc.scalar.copy(w[:, 0:2 * n], w32[:, 0:2 * n])
                last_wcast[0] = ci
                return w

            def wsl(wlist, n, kt, c0, c1):
                """lhsT slice [128, c0:c1] of k-tile kt from wide-tile list."""
                base = (kt % 2) * n
                return wlist[kt // 2][:, base + c0: base + c1]

            def load_bias(dram, o0, nparts, nm, tag="bias", bufs=16):
                b = pp.tile([128, 1], f32, tag=tag, bufs=bufs, name=nm)
                nc.gpsimd.dma_start(b[0:nparts, :], dram[o0:o0 + nparts, 0:1])
                return b

            # ============ Phase M1: text/image projections -> comb ============
            tbf = [pp.tile([128, BS], bf16, tag="ti", bufs=12, name=f"tbf_{ft}")
                   for ft in range(FT)]
            ibf = [pp.tile([128, BS], bf16, tag="ti", bufs=12, name=f"ibf_{ft}")
                   for ft in range(FT)]
            for src, dst in ((tT, tbf), (iT, ibf)):
                for ft in range(FT):
                    s32 = wk.tile([128, BS], f32, tag="s32", bufs=2, name="s32")
                    st.dma_start(s32[:], src[ft * 128:(ft + 1) * 128, :])
                    nc.scalar.copy(dst[ft][:], s32[:])

            wtb = [load_w2(WtT, p, P_DIM, f"wtb{p}") for p in range(FT // 2)]
            wib = [load_w2(WiT, p, P_DIM, f"wib{p}") for p in range(FT // 2)]
            btb = [load_bias(bt, mt * 128, 128, f"btb{mt}") for mt in range(4)]
            bib = [load_bias(bi, mt * 128, 128, f"bib{mt}") for mt in range(4)]

            comb = [pp.tile([128, BS], bf16, tag="cbp", bufs=PT2, name=f"comb_{d}")
                    for d in range(PT2)]
            for wsel, xsel, bsel, doff in ((wtb, tbf, btb, 0), (wib, ibf, bib, 4)):
                for mt in range(4):
                    for h in range(NH):
                        acc = psum()
                        for kt in range(FT):
                            nc.tensor.matmul(
                                acc[:], wsl(wsel, P_DIM, kt, mt * 128, (mt + 1) * 128),
                                xsel[kt][:, h * 512:(h + 1) * 512],
                                start=(kt == 0), stop=(kt == FT - 1))
                        nc.scalar.activation(
                            comb[doff + mt][:, h * 512:(h + 1) * 512], acc[:],
                            AF.Gelu, bias=bsel[mt][0:128, :])

            # ============ Phase M2: router ============
            rw1 = [load_w2(rtW1T, p, H_DIM // 4, f"rw1{p}") for p in range(PT2 // 2)]
            rb1 = [load_bias(rtb1, mt * 128, 128, f"rb1{mt}") for mt in range(2)]
            rh = [pp.tile([128, BS], bf16, tag="rh", bufs=2, name=f"rh_{mt}")
                  for mt in range(2)]
            for mt in range(2):
                for h in range(NH):
                    acc = psum()
                    for kt in range(PT2):
                        nc.tensor.matmul(
                            acc[:], wsl(rw1, H_DIM // 4, kt, mt * 128, (mt + 1) * 128),
                            comb[kt][:, h * 512:(h + 1) * 512],
                            start=(kt == 0), stop=(kt == PT2 - 1))
                    nc.scalar.activation(rh[mt][:, h * 512:(h + 1) * 512], acc[:],
                                         AF.Gelu, bias=rb1[mt][0:128, :])
            rw2 = [pp.tile([128, N_EXP], bf16, tag="wsmall", bufs=10, name=f"rw2{kt}")
                   for kt in range(2)]
            for kt in range(2):
                w32 = wk.tile([128, BS], f32, tag="s32", bufs=2, name="w32r2")
                ld.dma_start(w32[:, 0:N_EXP], rtW2T[kt * 128:(kt + 1) * 128, 0:N_EXP])
                nc.scalar.copy(rw2[kt][:], w32[:, 0:N_EXP])
            rb2 = load_bias(rtb2, 0, N_EXP, "rb2")
            rexp = pp.tile([N_EXP, BS], bf16, tag="rexp")
            for h in range(NH):
                acc4 = psum1("acc4")
                for kt in range(2):
                    nc.tensor.matmul(acc4[0:N_EXP, :], rw2[kt][:],
                                     rh[kt][:, h * 512:(h + 1) * 512],
                                     start=(kt == 0), stop=(kt == 1))
                # exp(logits + bias) in one ACT pass (softmax numerator)
                nc.scalar.activation(rexp[:, h * 512:(h + 1) * 512], acc4[0:N_EXP, :],
                                     AF.Exp, bias=rb2[0:N_EXP, :])
            rsum = [psum1(f"rsum{h}") for h in range(NH)]
            for h in range(NH):
                nc.tensor.matmul(rsum[h][0:1, :], ones[0:N_EXP, :],
                                 rexp[:, h * 512:(h + 1) * 512], start=True, stop=True)
            rs32 = vec1("rs32")
            for h in range(NH):
                nc.vector.tensor_copy(rs32[:, h * 512:(h + 1) * 512], rsum[h][0:1, :])
            rsinv = vec1("rsinv")
            nc.vector.reciprocal_approx_fast(rsinv[:], rs32[:])
            rsinvb = pp.tile([N_EXP, BS], f32, tag="bcast", bufs=2, name="rsinvb")
            nc.gpsimd.partition_broadcast(rsinvb[:], rsinv[:])
            rp = pp.tile([N_EXP, BS], f32, tag="rp")
            nc.vector.tensor_tensor(rp[:], rexp[:], rsinvb[:], OP.mult)
            st.dma_start(out_rp[:, :], rp[:])

            # ============ Phase M3: ds scalar MLP ============
            dw1 = [load_w2(dsW1T, p, H_DIM, f"dw1{p}") for p in range(PT2 // 2)]
            db1 = [load_bias(dsb1, mt * 128, 128, f"db1{mt}") for mt in range(HT)]
            dh = [pp.tile([128, BS], bf16, tag="hid", bufs=HT, name=f"dh_{mt}")
                  for mt in range(HT)]
            for mt in range(HT):
                for h in range(NH):
                    acc = psum()
                    for kt in range(PT2):
                        nc.tensor.matmul(
                            acc[:], wsl(dw1, H_DIM, kt, mt * 128, (mt + 1) * 128),
                            comb[kt][:, h * 512:(h + 1) * 512],
                            start=(kt == 0), stop=(kt == PT2 - 1))
                    nc.scalar.activation(dh[mt][:, h * 512:(h + 1) * 512], acc[:],
                                         AF.Gelu, bias=db1[mt][0:128, :])
            dw2 = [pp.tile([128, 1], bf16, tag="wsmall", bufs=10, name=f"dw2{kt}")
                   for kt in range(HT)]
            for kt in range(HT):
                w32 = wk.tile([128, BS], f32, tag="s32", bufs=2, name="w32d2")
                ld.dma_start(w32[:, 0:1], dsW2T[kt * 128:(kt + 1) * 128, 0:1])
                nc.scalar.copy(dw2[kt][:], w32[:, 0:1])
            db2 = load_bias(dsb2, 0, 1, "db2")
            ds = vec1("ds")
            for h in range(NH):
                acc1 = psum1("acc1")
                for kt in range(HT):
                    nc.tensor.matmul(acc1[0:1, :], dw2[kt][:],
                                     dh[kt][:, h * 512:(h + 1) * 512],
                                     start=(kt == 0), stop=(kt == HT - 1))
                nc.scalar.activation(ds[:, h * 512:(h + 1) * 512], acc1[0:1, :],
                                     AF.Sigmoid, bias=db2[0:1, :])
            st.dma_start(out_ds[:, :], ds[:])
            ds_bf = vec1("ds_bf", bf16)
            nc.vector.tensor_copy(ds_bf[:], ds[:])
            dsb = bcast128("dsb")
            nc.gpsimd.partition_broadcast(dsb[:], ds_bf[:])

            # combined_ti = image + ds*(text - image), computed in place:
            #   tbf <- tbf - ibf ; ibf <- ibf + dsb * tbf
            for ft in range(FT):
                nc.vector.tensor_tensor(tbf[ft][:], tbf[ft][:], ibf[ft][:], OP.subtract)
                dtm = wbf16("dtm")
                nc.vector.tensor_tensor(dtm[:], tbf[ft][:], dsb[:], OP.mult)
                nc.vector.tensor_tensor(ibf[ft][:], ibf[ft][:], dtm[:], OP.add)

            # ============ Phase M4: experts + weighted mix ============
            acc_fh = [pp.tile([128, BS], bf16, tag="hid", bufs=HT, name=f"accfh_{mt}")
                      for mt in range(HT)]
            # output-layer weights loaded before the experts so the
            # post-AllGather tail has no DMA dependency
            ow = [load_w2(outWT, p, F_DIM, f"ow{p}") for p in range(HT // 2)]
            # ============ Phase T: normalize targets, AllGather ============
            gb16 = [pp.tile([128, TS], bf16, tag="gb16", bufs=FT, name=f"gb16_{ft}")
                    for ft in range(FT)]
            gss = [psum1(f"gss{h}") for h in range(NH)]
            for ft in range(FT):
                s32 = wk.tile([128, BS], f32, tag="sg", bufs=2, name="s32g")
                ld.dma_start(s32[:], gT[ft * 128:(ft + 1) * 128, :])
                nc.vector.tensor_copy(gb16[ft][:], s32[:])
                gsq = wbf16("gsq")
                nc.vector.tensor_mul(gsq[:], gb16[ft][:], gb16[ft][:])
                for h in range(NH):
                    nc.tensor.matmul(gss[h][0:1, :], ones[:],
                                     gsq[:, h * 512:(h + 1) * 512],
                                     start=(ft == 0), stop=(ft == FT - 1))
            gn = vec1("gn")
            for h in range(NH):
                nc.scalar.activation(gn[:, h * 512:(h + 1) * 512], gss[h][0:1, :],
                                     AF.Sqrt)
            gni = vec1("gni")
            nc.vector.reciprocal_approx_fast(gni[:], gn[:])
            gnib = bcast128("gnib", f32)
            nc.gpsimd.partition_broadcast(gnib[:], gni[:])
            for ft in range(FT):
                gbf = wbf16("gbf")
                nc.vector.tensor_tensor(gbf[:], gb16[ft][:], gnib[:], OP.mult)
                nc.gpsimd.dma_start(agin[ft * 128:(ft + 1) * 128, :], gbf[:])
            # The AllGather monopolizes the SDMA engines for its duration
            # (~80us blackout for regular DMA). Fire it only once the weights
            # for experts 0-1 are resident so the PE has enough buffered work
            # to ride out the blackout; the dependency is attached after the
            # expert loads below.
            ag_inst = nc.gpsimd.collective_compute(
                "AllGather", OP.bypass,
                replica_groups=[list(range(N_CORES))],
                ins=[agin.ap().opt()],
                outs=[agout.ap().opt()],
            )

            lam_sb = constp.tile([1, 1], f32)
            ld.dma_start(lam_sb[:], lam[0:1, 0:1])
            em2l = constp.tile([1, 1], f32)
            nc.scalar.activation(em2l[:], lam_sb[:], AF.Exp, scale=-2.0)


            from concourse.tile import add_dep_helper
            for e in range(N_EXP):
                we = [load_w2(expWT[e], p, H_DIM, f"we{e}_{p}") for p in range(PT2 // 2)]
                if e == 3:
                    add_dep_helper(ag_inst.ins, last_wcast[0].ins, True,
                                   "delay AllGather until all expert weights resident")
                eb = [load_bias(expb, e * H_DIM + mt * 128, 128, f"eb{e}_{mt}")
                      for mt in range(HT)]
                # rp row -> partition 0 -> bf16 -> broadcast to 128 partitions
                rp0 = pp.tile([1, BS], f32, tag="rp0", bufs=1, name=f"rp0_{e}")
                ld.dma_start(rp0[:], rp[e:e + 1, :])
                rp0b = pp.tile([1, BS], bf16, tag="rp0b", bufs=1, name=f"rp0b_{e}")
                nc.vector.tensor_copy(rp0b[:], rp0[:])
                rpbe = bcast128(f"rpb{e}")
                nc.gpsimd.partition_broadcast(rpbe[:], rp0b[:])
                for mt in range(HT):
                    eo = wbf16("eo")
                    for h in range(NH):
                        acc = psum()
                        for kt in range(PT2):
                            nc.tensor.matmul(
                                acc[:], wsl(we, H_DIM, kt, mt * 128, (mt + 1) * 128),
                                comb[kt][:, h * 512:(h + 1) * 512],
                                start=(kt == 0), stop=(kt == PT2 - 1))
                        nc.scalar.activation(eo[:, h * 512:(h + 1) * 512], acc[:],
                                             AF.Gelu, bias=eb[mt][0:128, :])
                    if e == 0:
                        nc.vector.tensor_tensor(acc_fh[mt][:], eo[:], rpbe[:], OP.mult)
                    else:
                        tmp = wbf16("etmp")
                        nc.vector.tensor_tensor(tmp[:], eo[:], rpbe[:], OP.mult)
                        nc.vector.tensor_tensor(acc_fh[mt][:], acc_fh[mt][:], tmp[:],
                                                OP.add)

            # ============ Phase M5: output layer + combine ============
            ob = [load_bias(outb, mt * 128, 128, f"ob{mt}") for mt in range(FT)]
            o32 = [pp.tile([128, BS], bf16, tag="cbp", bufs=PT2, name=f"o32_{ft}")
                   for ft in range(FT)]
            for mt in range(FT):
                for h in range(NH):
                    acc = psum()
                    for kt in range(HT):
                        nc.tensor.matmul(
                            acc[:], wsl(ow, F_DIM, kt, mt * 128, (mt + 1) * 128),
                            acc_fh[kt][:, h * 512:(h + 1) * 512],
                            start=(kt == 0), stop=(kt == HT - 1))
                    # o32 = (psum + outb) + combined_ti  in one DVE pass
                    nc.vector.scalar_tensor_tensor(
                        o32[mt][:, h * 512:(h + 1) * 512], acc[:], ob[mt][0:128, :],
                        ibf[mt][:, h * 512:(h + 1) * 512], OP.add, OP.add)

            # per-batch inverse norm (scaled by exp(logit_scale)) in BATCH-major
            # layout [128, HT]: invbm[p, mt] = exp(lam)/||out_{mt*128+p}||
            oss = [psum1(f"oss{h}") for h in range(NH)]
            for ft in range(FT):
                osq = wbf16("osq")
                nc.scalar.activation(osq[:], o32[ft][:], AF.Square)
                for h in range(NH):
                    nc.tensor.matmul(oss[h][0:1, :], ones[:],
                                     osq[:, h * 512:(h + 1) * 512],
                                     start=(ft == 0), stop=(ft == FT - 1))
            onrm = vec1("onrm")
            for h in range(NH):
                nc.scalar.activation(onrm[:, h * 512:(h + 1) * 512], oss[h][0:1, :],
                                     AF.Sqrt, scale=em2l[:])
            oinv = vec1("oinv")
            nc.vector.reciprocal_approx_fast(oinv[:], onrm[:])
            st.dma_start(invbounce[0:1, :], oinv[0:1, :])
            invbm = pp.tile([128, HT], f32, tag="invbm")
            ld.dma_start(invbm[:],
                         invbounce.ap().rearrange("o (m p) -> (o p) m", p=128))

            # ============ Phase L: logits = out.T @ gathered, scaled ============
            for rb in range(N_CORES):
                for h in range(NH):
                    rtw = pp.tile([128, FT * 512], bf16, tag="rtw", bufs=2,
                                  name=f"rtw{rb}_{h}")
                    src = agout[rb * F_DIM:(rb + 1) * F_DIM, h * 512:(h + 1) * 512]
                    ld.dma_start(rtw[:].rearrange("p (k c) -> p k c", k=FT),
                                 src.rearrange("(k p) c -> p k c", p=128))
                    for mtg in range(4):
                        lt = wk.tile([128, 2 * 512], f32, tag="lt", bufs=2, name="lt")
                        for j in range(2):
                            mt = mtg * 2 + j
                            acc = psum()
                            for kt in range(FT):
                                nc.tensor.matmul(
                                    acc[:], o32[kt][:, mt * 128:(mt + 1) * 128],
                                    rtw[:, kt * 512:(kt + 1) * 512],
                                    start=(kt == 0), stop=(kt == FT - 1))
                            # scale by exp(lam)/||out_b|| during PSUM->SBUF
                            if mt % 2 == 0:
                                nc.vector.tensor_scalar_mul(
                                    lt[:, j * 512:(j + 1) * 512], acc[:],
                                    invbm[:, mt:mt + 1])
                            else:
                                nc.scalar.activation(
                                    lt[:, j * 512:(j + 1) * 512], acc[:],
                                    AF.Copy, scale=invbm[:, mt:mt + 1])
                        dst = out_logits[mtg * 256:(mtg + 1) * 256,
                                         rb * TS + h * 512: rb * TS + (h + 1) * 512]
                        st.dma_start(dst.rearrange("(j p) c -> p j c", p=128),
                                     lt[:].rearrange("p (j c) -> p j c", j=2))
    return nc


_CACHED = {}


def _get_compiled():
    if "nc" not in _CACHED:
        nc = build()
        nc.compile()
        _CACHED["nc"] = nc
    return _CACHED["nc"]


def kernel(image_features, text_features, target_features, Wt, bt, Wi, bi,
           ds_W1, ds_b1, ds_W2, ds_b2, exp_W, exp_b,
           rt_W1, rt_b1, rt_W2, rt_b2, out_W, out_b, logit_scale):
    from concourse.bass_utils import run_bass_kernel_spmd

    f = np.float32
    c = np.ascontiguousarray
    tTa = c(np.asarray(text_features, f).T)      # [768, 8192]
    iTa = c(np.asarray(image_features, f).T)
    gTa = c(np.asarray(target_features, f).T)

    common = {
        "WtT": c(np.asarray(Wt, f).T), "WiT": c(np.asarray(Wi, f).T),
        "bt": np.asarray(bt, f).reshape(-1, 1), "bi": np.asarray(bi, f).reshape(-1, 1),
        "rtW1T": c(np.asarray(rt_W1, f).T), "rtb1": np.asarray(rt_b1, f).reshape(-1, 1),
        "rtW2T": c(np.asarray(rt_W2, f).T), "rtb2": np.asarray(rt_b2, f).reshape(-1, 1),
        "dsW1T": c(np.asarray(ds_W1, f).T), "dsb1": np.asarray(ds_b1, f).reshape(-1, 1),
        "dsW2T": c(np.asarray(ds_W2, f).T), "dsb2": np.asarray(ds_b2, f).reshape(-1, 1),
        "expWT": c(np.asarray(exp_W, f).transpose(0, 2, 1)),
        "expb": np.asarray(exp_b, f).reshape(-1, 1),
        "outWT": c(np.asarray(out_W, f).T), "outb": np.asarray(out_b, f).reshape(-1, 1),
        "lam": np.asarray(logit_scale, f).reshape(1, 1),
    }
    in_maps = []
    for r in range(N_CORES):
        sl = slice(r * BS, (r + 1) * BS)
        in_maps.append({
            "tT": c(tTa[:, sl]), "iT": c(iTa[:, sl]), "gT": c(gTa[:, sl]), **common,
        })

    nc = _get_compiled()
    res = run_bass_kernel_spmd(nc, in_maps, core_ids=list(range(N_CORES)))

    logits = np.concatenate([res.results[r]["out_logits"] for r in range(N_CORES)],
                            axis=0)
    ds = np.concatenate([res.results[r]["out_ds"][0] for r in range(N_CORES)])[:, None]
    rp = np.concatenate([res.results[r]["out_rp"].T for r in range(N_CORES)], axis=0)
    return logits, ds, rp


# revision 30
# speedup vs baseline: 1.1163x; 1.1163x over previous
"""AdaFuse MoE-routing kernel for 8 Trainium2 NeuronCores (Bass/Tile).

Strategy (pure data parallel over batch B=8192, 1024 rows per core):
  - All activations kept FEATURE-MAJOR on-chip ([feat_part, batch_free]) so
    every Linear layer is out = W_T.T @ act with no per-layer transposes;
    host passes inputs/weights pre-transposed (layout choice only).
  - Matmuls run in bf16 (f32 PSUM accumulation); elementwise work in f32/bf16.
  - Each core normalizes its target shard, casts bf16, AllGathers the
    [768,1024] shard -> [6144,1024] (rank-major blocks), then computes its
    [1024, 8192] logits block streaming gathered tiles.
  - The output normalization (1/||out|| * exp(logit_scale)) is applied in the
    logits PSUM->SBUF epilogue as a per-partition scale, so logits matmuls
    depend only on the unnormalized output tiles.
  - Weight loads / gathered-target loads / logits stores are batched into
    wide DMAs (DMA-issue on the sequencers costs ~0.7us per descriptor).

Self-contained: hardcodes all shapes; no file reads.
"""
import sys

sys.path.insert(0, "/opt/trn_rl_repo")

import numpy as np

F_DIM, P_DIM, H_DIM, N_EXP = 768, 512, 1024, 4
B, T = 8192, 8192
N_CORES = 8
BS = B // N_CORES            # 1024 batch rows per core
TS = T // N_CORES            # 1024 target rows per core
FT = F_DIM // 128            # 6 feature tiles
PT2 = (2 * P_DIM) // 128     # 8 comb-dim tiles
HT = H_DIM // 128            # 8 hidden tiles
NH = BS // 512               # 2 free-dim halves per batch


def build():
    from concourse import bacc, tile, mybir

    f32 = mybir.dt.float32
    bf16 = mybir.dt.bfloat16
    AF = mybir.ActivationFunctionType
    OP = mybir.AluOpType

    nc = bacc.Bacc(None, target_bir_lowering=False, debug=False, num_devices=N_CORES)

    # ---- kernel I/O ----
    tT = nc.dram_tensor("tT", [F_DIM, BS], f32, kind="ExternalInput")
    iT = nc.dram_tensor("iT", [F_DIM, BS], f32, kind="ExternalInput")
    gT = nc.dram_tensor("gT", [F_DIM, TS], f32, kind="ExternalInput")
    WtT = nc.dram_tensor("WtT", [F_DIM, P_DIM], f32, kind="ExternalInput")
    WiT = nc.dram_tensor("WiT", [F_DIM, P_DIM], f32, kind="ExternalInput")
    bt = nc.dram_tensor("bt", [P_DIM, 1], f32, kind="ExternalInput")
    bi = nc.dram_tensor("bi", [P_DIM, 1], f32, kind="ExternalInput")
    rtW1T = nc.dram_tensor("rtW1T", [2 * P_DIM, H_DIM // 4], f32, kind="ExternalInput")
    rtb1 = nc.dram_tensor("rtb1", [H_DIM // 4, 1], f32, kind="ExternalInput")
    rtW2T = nc.dram_tensor("rtW2T", [H_DIM // 4, N_EXP], f32, kind="ExternalInput")
    rtb2 = nc.dram_tensor("rtb2", [N_EXP, 1], f32, kind="ExternalInput")
    dsW1T = nc.dram_tensor("dsW1T", [2 * P_DIM, H_DIM], f32, kind="ExternalInput")
    dsb1 = nc.dram_tensor("dsb1", [H_DIM, 1], f32, kind="ExternalInput")
    dsW2T = nc.dram_tensor("dsW2T", [H_DIM, 1], f32, kind="ExternalInput")
    dsb2 = nc.dram_tensor("dsb2", [1, 1], f32, kind="ExternalInput")
    expWT = nc.dram_tensor("expWT", [N_EXP, 2 * P_DIM, H_DIM], f32, kind="ExternalInput")
    expb = nc.dram_tensor("expb", [N_EXP * H_DIM, 1], f32, kind="ExternalInput")
    outWT = nc.dram_tensor("outWT", [H_DIM, F_DIM], f32, kind="ExternalInput")
    outb = nc.dram_tensor("outb", [F_DIM, 1], f32, kind="ExternalInput")
    lam = nc.dram_tensor("lam", [1, 1], f32, kind="ExternalInput")

    out_logits = nc.dram_tensor("out_logits", [BS, T], f32, kind="ExternalOutput")
    out_ds = nc.dram_tensor("out_ds", [1, BS], f32, kind="ExternalOutput")
    out_rp = nc.dram_tensor("out_rp", [N_EXP, BS], f32, kind="ExternalOutput")

    agin = nc.dram_tensor("agin", [F_DIM, TS], bf16, kind="Internal")
    agout = nc.dram_tensor("agout", [N_CORES * F_DIM, TS], bf16,
                           kind="Internal", addr_space="Shared")
    invbounce = nc.dram_tensor("invbounce", [1, HT * 128], f32, kind="Internal")

    ld = nc.sync       # input DMA queue (HWDGE)
    st = nc.scalar     # output / secondary DMA queue (HWDGE)

    with tile.TileContext(nc) as tc:
        with (
            tc.tile_pool(name="const", bufs=1) as constp,
            tc.tile_pool(name="pp", bufs=1) as pp,
            tc.tile_pool(name="wk", bufs=1) as wk,
            tc.tile_pool(name="ps", bufs=6, space="PSUM") as ps,
            tc.tile_pool(name="ps1", bufs=2, space="PSUM") as ps1,
        ):
            ones = constp.tile([128, 1], bf16)
            nc.vector.memset(ones[:], 1.0)
            def psum(tag="mm"):
                return ps.tile([128, 512], f32, tag=tag, bufs=6, name=tag)

            def psum1(nm):
                return ps1.tile([N_EXP, 512], f32, tag="ss", bufs=2, name=nm)

            def vec1(nm, dt=f32):
                return pp.tile([1, BS], dt, tag="vec1", bufs=2, name=nm)

            def bcast128(nm, dt=bf16):
                return pp.tile([128, BS], dt, tag="bcast", bufs=2, name=nm)

            def wbf16(nm, tag="wbf", bufs=2):
                return wk.tile([128, BS], bf16, tag=tag, bufs=bufs, name=nm)

            last_wcast = [None]

            def load_w2(dram2d, pair, n, nm):
                """Two adjacent [128, n] f32 k-tiles -> one wide bf16 tile.

                Returns tile whose cols [j*n:(j+1)*n] hold k-tile 2*pair+j.
                """
                w32 = wk.tile([128, 2048], f32, tag="w32", bufs=2, name="w32" + nm)
                src = dram2d[pair * 256:(pair + 1) * 256, 0:n]
                src = src.rearrange("(j p) c -> p j c", p=128)
                eng = ld if (pair % 2 == 0) else nc.gpsimd
                eng.dma_start(w32[:, 0:2 * n].rearrange("p (j c) -> p j c", j=2), src)
                w = pp.tile([128, 2048], bf16, tag="wbig", bufs=12, name=nm)
                if n >= 768:
                    ci = nc.vector.tensor_copy(w[:, 0:2 * n], w32[:, 0:2 * n])
                else:
                    ci = nc.scalar.copy(w[:, 0:2 * n], w32[:, 0:2 * n])
                last_wcast[0] = ci
                return w

            def wsl(wlist, n, kt, c0, c1):
                """lhsT slice [128, c0:c1] of k-tile kt from wide-tile list."""
                base = (kt % 2) * n
                return wlist[kt // 2][:, base + c0: base + c1]

            def load_bias(dram, o0, nparts, nm, tag="bias", bufs=16):
                b = pp.tile([128, 1], f32, tag=tag, bufs=bufs, name=nm)
                nc.gpsimd.dma_start(b[0:nparts, :], dram[o0:o0 + nparts, 0:1])
                return b

            # ============ Phase T: normalize targets, AllGather ============
            gb16 = [pp.tile([128, TS], bf16, tag="gb16", bufs=FT, name=f"gb16_{ft}")
                    for ft in range(FT)]
            gss = [psum1(f"gss{h}") for h in range(NH)]
            for ft in range(FT):
                s32 = wk.tile([128, BS], f32, tag="sg", bufs=2, name="s32g")
                ld.dma_start(s32[:], gT[ft * 128:(ft + 1) * 128, :])
                nc.vector.tensor_copy(gb16[ft][:], s32[:])
                gsq = wbf16("gsq")
                nc.vector.tensor_mul(gsq[:], gb16[ft][:], gb16[ft][:])
                for h in range(NH):
                    nc.tensor.matmul(gss[h][0:1, :], ones[:],
                                     gsq[:, h * 512:(h + 1) * 512],
                                     start=(ft == 0), stop=(ft == FT - 1))
            gn = vec1("gn")
            for h in range(NH):
                nc.scalar.activation(gn[:, h * 512:(h + 1) * 512], gss[h][0:1, :],
                                     AF.Sqrt)
            gni = vec1("gni")
            nc.vector.reciprocal_approx_fast(gni[:], gn[:])
            gnib = bcast128("gnib", f32)
            nc.gpsimd.partition_broadcast(gnib[:], gni[:])
            for ft in range(FT):
                gbf = wbf16("gbf")
                nc.vector.tensor_tensor(gbf[:], gb16[ft][:], gnib[:], OP.mult)
                nc.gpsimd.dma_start(agin[ft * 128:(ft + 1) * 128, :], gbf[:])
            # The AllGather monopolizes the SDMA engines for its duration
            # (~80us blackout for regular DMA). Fire it only once the weights
            # for experts 0-1 are resident so the PE has enough buffered work
            # to ride out the blackout; the dependency is attached after the
            # expert loads below.
            ag_inst = nc.gpsimd.collective_compute(
                "AllGather", OP.bypass,
                replica_groups=[list(range(N_CORES))],
                ins=[agin.ap().opt()],
                outs=[agout.ap().opt()],
            )

            lam_sb = constp.tile([1, 1], f32)
            ld.dma_start(lam_sb[:], lam[0:1, 0:1])
            em2l = constp.tile([1, 1], f32)
            nc.scalar.activation(em2l[:], lam_sb[:], AF.Exp, scale=-2.0)



            # ============ Phase M1: text/image projections -> comb ============
            tbf = [pp.tile([128, BS], bf16, tag="ti", bufs=12, name=f"tbf_{ft}")
                   for ft in range(FT)]
            ibf = [pp.tile([128, BS], bf16, tag="ti", bufs=12, name=f"ibf_{ft}")
                   for ft in range(FT)]
            for src, dst in ((tT, tbf), (iT, ibf)):
                for ft in range(FT):
                    s32 = wk.tile([128, BS], f32, tag="s32", bufs=2, name="s32")
                    st.dma_start(s32[:], src[ft * 128:(ft + 1) * 128, :])
                    nc.scalar.copy(dst[ft][:], s32[:])

            wtb = [load_w2(WtT, p, P_DIM, f"wtb{p}") for p in range(FT // 2)]
            wib = [load_w2(WiT, p, P_DIM, f"wib{p}") for p in range(FT // 2)]
            btb = [load_bias(bt, mt * 128, 128, f"btb{mt}") for mt in range(4)]
            bib = [load_bias(bi, mt * 128, 128, f"bib{mt}") for mt in range(4)]

            comb = [pp.tile([128, BS], bf16, tag="cbp", bufs=PT2, name=f"comb_{d}")
                    for d in range(PT2)]
            for wsel, xsel, bsel, doff in ((wtb, tbf, btb, 0), (wib, ibf, bib, 4)):
                for mt in range(4):
                    for h in range(NH):
                        acc = psum()
                        for kt in range(FT):
                            nc.tensor.matmul(
                                acc[:], wsl(wsel, P_DIM, kt, mt * 128, (mt + 1) * 128),
                                xsel[kt][:, h * 512:(h + 1) * 512],
                                start=(kt == 0), stop=(kt == FT - 1))
                        nc.scalar.activation(
                            comb[doff + mt][:, h * 512:(h + 1) * 512], acc[:],
                            AF.Gelu, bias=bsel[mt][0:128, :])

            # ============ Phase M2: router ============
            rw1 = [load_w2(rtW1T, p, H_DIM // 4, f"rw1{p}") for p in range(PT2 // 2)]
            rb1 = [load_bias(rtb1, mt * 128, 128, f"rb1{mt}") for mt in range(2)]
            rh = [pp.tile([128, BS], bf16, tag="rh", bufs=2, name=f"rh_{mt}")
                  for mt in range(2)]
            for mt in range(2):
                for h in range(NH):
                    acc = psum()
                    for kt in range(PT2):
                        nc.tensor.matmul(
                            acc[:], wsl(rw1, H_DIM // 4, kt, mt * 128, (mt + 1) * 128),
                            comb[kt][:, h * 512:(h + 1) * 512],
                            start=(kt == 0), stop=(kt == PT2 - 1))
                    nc.scalar.activation(rh[mt][:, h * 512:(h + 1) * 512], acc[:],
                                         AF.Gelu, bias=rb1[mt][0:128, :])
            rw2 = [pp.tile([128, N_EXP], bf16, tag="wsmall", bufs=10, name=f"rw2{kt}")
                   for kt in range(2)]
            for kt in range(2):
                w32 = wk.tile([128, BS], f32, tag="s32", bufs=2, name="w32r2")
                ld.dma_start(w32[:, 0:N_EXP], rtW2T[kt * 128:(kt + 1) * 128, 0:N_EXP])
                nc.scalar.copy(rw2[kt][:], w32[:, 0:N_EXP])
            rb2 = load_bias(rtb2, 0, N_EXP, "rb2")
            rexp = pp.tile([N_EXP, BS], bf16, tag="rexp")
            for h in range(NH):
                acc4 = psum1("acc4")
                for kt in range(2):
                    nc.tensor.matmul(acc4[0:N_EXP, :], rw2[kt][:],
                                     rh[kt][:, h * 512:(h + 1) * 512],
                                     start=(kt == 0), stop=(kt == 1))
                # exp(logits + bias) in one ACT pass (softmax numerator)
                nc.scalar.activation(rexp[:, h * 512:(h + 1) * 512], acc4[0:N_EXP, :],
                                     AF.Exp, bias=rb2[0:N_EXP, :])
            rsum = [psum1(f"rsum{h}") for h in range(NH)]
            for h in range(NH):
                nc.tensor.matmul(rsum[h][0:1, :], ones[0:N_EXP, :],
                                 rexp[:, h * 512:(h + 1) * 512], start=True, stop=True)
            rs32 = vec1("rs32")
            for h in range(NH):
                nc.vector.tensor_copy(rs32[:, h * 512:(h + 1) * 512], rsum[h][0:1, :])
            rsinv = vec1("rsinv")
            nc.vector.reciprocal_approx_fast(rsinv[:], rs32[:])
            rsinvb = pp.tile([N_EXP, BS], f32, tag="bcast", bufs=2, name="rsinvb")
            nc.gpsimd.partition_broadcast(rsinvb[:], rsinv[:])
            rp = pp.tile([N_EXP, BS], f32, tag="rp")
            nc.vector.tensor_tensor(rp[:], rexp[:], rsinvb[:], OP.mult)
            st.dma_start(out_rp[:, :], rp[:])

            # ============ Phase M3: ds scalar MLP ============
            dw1 = [load_w2(dsW1T, p, H_DIM, f"dw1{p}") for p in range(PT2 // 2)]
            db1 = [load_bias(dsb1, mt * 128, 128, f"db1{mt}") for mt in range(HT)]
            dh = [pp.tile([128, BS], bf16, tag="hid", bufs=HT, name=f"dh_{mt}")
                  for mt in range(HT)]
            for mt in range(HT):
                for h in range(NH):
                    acc = psum()
                    for kt in range(PT2):
                        nc.tensor.matmul(
                            acc[:], wsl(dw1, H_DIM, kt, mt * 128, (mt + 1) * 128),
                            comb[kt][:, h * 512:(h + 1) * 512],
                            start=(kt == 0), stop=(kt == PT2 - 1))
                    nc.scalar.activation(dh[mt][:, h * 512:(h + 1) * 512], acc[:],
                                         AF.Gelu, bias=db1[mt][0:128, :])
            dw2 = [pp.tile([128, 1], bf16, tag="wsmall", bufs=10, name=f"dw2{kt}")
                   for kt in range(HT)]
            for kt in range(HT):
                w32 = wk.tile([128, BS], f32, tag="s32", bufs=2, name="w32d2")
                ld.dma_start(w32[:, 0:1], dsW2T[kt * 128:(kt + 1) * 128, 0:1])
                nc.scalar.copy(dw2[kt][:], w32[:, 0:1])
            db2 = load_bias(dsb2, 0, 1, "db2")
            ds = vec1("ds")
            for h in range(NH):
                acc1 = psum1("acc1")
                for kt in range(HT):
                    nc.tensor.matmul(acc1[0:1, :], dw2[kt][:],
                                     dh[kt][:, h * 512:(h + 1) * 512],
                                     start=(kt == 0), stop=(kt == HT - 1))
                nc.scalar.activation(ds[:, h * 512:(h + 1) * 512], acc1[0:1, :],
                                     AF.Sigmoid, bias=db2[0:1, :])
            st.dma_start(out_ds[:, :], ds[:])
            ds_bf = vec1("ds_bf", bf16)
            nc.vector.tensor_copy(ds_bf[:], ds[:])
            dsb = bcast128("dsb")
            nc.gpsimd.partition_broadcast(dsb[:], ds_bf[:])

            # combined_ti = image + ds*(text - image), computed in place:
            #   tbf <- tbf - ibf ; ibf <- ibf + dsb * tbf
            for ft in range(FT):
                nc.vector.tensor_tensor(tbf[ft][:], tbf[ft][:], ibf[ft][:], OP.subtract)
                dtm = wbf16("dtm")
                nc.vector.tensor_tensor(dtm[:], tbf[ft][:], dsb[:], OP.mult)
                nc.vector.tensor_tensor(ibf[ft][:], ibf[ft][:], dtm[:], OP.add)

            # ============ Phase M4: experts + weighted mix ============
            acc_fh = [pp.tile([128, BS], bf16, tag="hid", bufs=HT, name=f"accfh_{mt}")
                      for mt in range(HT)]
            # output-layer weights loaded before the experts so the
            # post-AllGather tail has no DMA dependency
            ow = [load_w2(outWT, p, F_DIM, f"ow{p}") for p in range(HT // 2)]
            from concourse.tile import add_dep_helper
            for e in range(N_EXP):
                we = [load_w2(expWT[e], p, H_DIM, f"we{e}_{p}") for p in range(PT2 // 2)]
                if e == 3:
                    add_dep_helper(ag_inst.ins, last_wcast[0].ins, True,
                                   "delay AllGather until all expert weights resident")
                eb = [load_bias(expb, e * H_DIM + mt * 128, 128, f"eb{e}_{mt}")
                      for mt in range(HT)]
                # rp row -> partition 0 -> bf16 -> broadcast to 128 partitions
                rp0 = pp.tile([1, BS], f32, tag="rp0", bufs=1, name=f"rp0_{e}")
                ld.dma_start(rp0[:], rp[e:e + 1, :])
                rp0b = pp.tile([1, BS], bf16, tag="rp0b", bufs=1, name=f"rp0b_{e}")
                nc.vector.tensor_copy(rp0b[:], rp0[:])
                rpbe = bcast128(f"rpb{e}")
                nc.gpsimd.partition_broadcast(rpbe[:], rp0b[:])
                for mt in range(HT):
                    eo = wbf16("eo")
                    for h in range(NH):
                        acc = psum()
                        for kt in range(PT2):
                            nc.tensor.matmul(
                                acc[:], wsl(we, H_DIM, kt, mt * 128, (mt + 1) * 128),
                                comb[kt][:, h * 512:(h + 1) * 512],
                                start=(kt == 0), stop=(kt == PT2 - 1))
                        nc.scalar.activation(eo[:, h * 512:(h + 1) * 512], acc[:],
                                             AF.Gelu, bias=eb[mt][0:128, :])
                    if e == 0:
                        nc.vector.tensor_tensor(acc_fh[mt][:], eo[:], rpbe[:], OP.mult)
                    else:
                        tmp = wbf16("etmp")
                        nc.vector.tensor_tensor(tmp[:], eo[:], rpbe[:], OP.mult)
                        nc.vector.tensor_tensor(acc_fh[mt][:], acc_fh[mt][:], tmp[:],
                                                OP.add)

            # ============ Phase M5: output layer + combine ============
            ob = [load_bias(outb, mt * 128, 128, f"ob{mt}") for mt in range(FT)]
            o32 = [pp.tile([128, BS], bf16, tag="cbp", bufs=PT2, name=f"o32_{ft}")
                   for ft in range(FT)]
            for mt in range(FT):
                for h in range(NH):
                    acc = psum()
                    for kt in range(HT):
                        nc.tensor.matmul(
                            acc[:], wsl(ow, F_DIM, kt, mt * 128, (mt + 1) * 128),
                            acc_fh[kt][:, h * 512:(h + 1) * 512],
                            start=(kt == 0), stop=(kt == HT - 1))
                    # o32 = (psum + outb) + combined_ti  in one DVE pass
                    nc.vector.scalar_tensor_tensor(
                        o32[mt][:, h * 512:(h + 1) * 512], acc[:], ob[mt][0:128, :],
                        ibf[mt][:, h * 512:(h + 1) * 512], OP.add, OP.add)

            # per-batch inverse norm (scaled by exp(logit_scale)) in BATCH-major
            # layout [128, HT]: invbm[p, mt] = exp(lam)/||out_{mt*128+p}||
            oss = [psum1(f"oss{h}") for h in range(NH)]
            for ft in range(FT):
                osq = wbf16("osq")
                nc.scalar.activation(osq[:], o32[ft][:], AF.Square)
                for h in range(NH):
                    nc.tensor.matmul(oss[h][0:1, :], ones[:],
                                     osq[:, h * 512:(h + 1) * 512],
                                     start=(ft == 0), stop=(ft == FT - 1))
            onrm = vec1("onrm")
            for h in range(NH):
                nc.scalar.activation(onrm[:, h * 512:(h + 1) * 512], oss[h][0:1, :],
                                     AF.Sqrt, scale=em2l[:])
            oinv = vec1("oinv")
            nc.vector.reciprocal_approx_fast(oinv[:], onrm[:])
            st.dma_start(invbounce[0:1, :], oinv[0:1, :])
            invbm = pp.tile([128, HT], f32, tag="invbm")
            ld.dma_start(invbm[:],
                         invbounce.ap().rearrange("o (m p) -> (o p) m", p=128))

            # ============ Phase L: logits = out.T @ gathered, scaled ============
            for rb in range(N_CORES):
                for h in range(NH):
                    rtw = pp.tile([128, FT * 512], bf16, tag="rtw", bufs=2,
                                  name=f"rtw{rb}_{h}")
                    src = agout[rb * F_DIM:(rb + 1) * F_DIM, h * 512:(h + 1) * 512]
                    ld.dma_start(rtw[:].rearrange("p (k c) -> p k c", k=FT),
                                 src.rearrange("(k p) c -> p k c", p=128))
                    for mtg in range(4):
                        lt = wk.tile([128, 2 * 512], f32, tag="lt", bufs=2, name="lt")
                        for j in range(2):
                            mt = mtg * 2 + j
                            acc = psum()
                            for kt in range(FT):
                                nc.tensor.matmul(
                                    acc[:], o32[kt][:, mt * 128:(mt + 1) * 128],
                                    rtw[:, kt * 512:(kt + 1) * 512],
                                    start=(kt == 0), stop=(kt == FT - 1))
                            # scale by exp(lam)/||out_b|| during PSUM->SBUF
                            if mt % 2 == 0:
                                nc.vector.tensor_scalar_mul(
                                    lt[:, j * 512:(j + 1) * 512], acc[:],
                                    invbm[:, mt:mt + 1])
                            else:
                                nc.scalar.activation(
                                    lt[:, j * 512:(j + 1) * 512], acc[:],
                                    AF.Copy, scale=invbm[:, mt:mt + 1])
                        dst = out_logits[mtg * 256:(mtg + 1) * 256,
                                         rb * TS + h * 512: rb * TS + (h + 1) * 512]
                        st.dma_start(dst.rearrange("(j p) c -> p j c", p=128),
                                     lt[:].rearrange("p (j c) -> p j c", j=2))
    return nc


_CACHED = {}


def _get_compiled():
    if "nc" not in _CACHED:
        nc = build()
        nc.compile()
        _CACHED["nc"] = nc
    return _CACHED["nc"]


def kernel(image_features, text_features, target_features, Wt, bt, Wi, bi,
           ds_W1, ds_b1, ds_W2, ds_b2, exp_W, exp_b,
           rt_W1, rt_b1, rt_W2, rt_b2, out_W, out_b, logit_scale):
    from concourse.bass_utils import run_bass_kernel_spmd

    f = np.float32
    c = np.ascontiguousarray
    tTa = c(np.asarray(text_features, f).T)      # [768, 8192]
    iTa = c(np.asarray(image_features, f).T)
    gTa = c(np.asarray(target_features, f).T)

    common = {
        "WtT": c(np.asarray(Wt, f).T), "WiT": c(np.asarray(Wi, f).T),
        "bt": np.asarray(bt, f).reshape(-1, 1), "bi": np.asarray(bi, f).reshape(-1, 1),
        "rtW1T": c(np.asarray(rt_W1, f).T), "rtb1": np.asarray(rt_b1, f).reshape(-1, 1),
        "rtW2T": c(np.asarray(rt_W2, f).T), "rtb2": np.asarray(rt_b2, f).reshape(-1, 1),
        "dsW1T": c(np.asarray(ds_W1, f).T), "dsb1": np.asarray(ds_b1, f).reshape(-1, 1),
        "dsW2T": c(np.asarray(ds_W2, f).T), "dsb2": np.asarray(ds_b2, f).reshape(-1, 1),
        "expWT": c(np.asarray(exp_W, f).transpose(0, 2, 1)),
        "expb": np.asarray(exp_b, f).reshape(-1, 1),
        "outWT": c(np.asarray(out_W, f).T), "outb": np.asarray(out_b, f).reshape(-1, 1),
        "lam": np.asarray(logit_scale, f).reshape(1, 1),
    }
    in_maps = []
    for r in range(N_CORES):
        sl = slice(r * BS, (r + 1) * BS)
        in_maps.append({
            "tT": c(tTa[:, sl]), "iT": c(iTa[:, sl]), "gT": c(gTa[:, sl]), **common,
        })

    nc = _get_compiled()
    res = run_bass_kernel_spmd(nc, in_maps, core_ids=list(range(N_CORES)))

    logits = np.concatenate([res.results[r]["out_logits"] for r in range(N_CORES)],
                            axis=0)
    ds = np.concatenate([res.results[r]["out_ds"][0] for r in range(N_CORES)])[:, None]
    rp = np.concatenate([res.results[r]["out_rp"].T for r in range(N_CORES)], axis=0)
    return logits, ds, rp
